# revision 20
# baseline (speedup 1.0000x reference)
"""Triplet-margin loss (EuclideanTriple) on 8 Trainium2 NeuronCores.

loss = sum_i relu( ||x_i - y_i + eps||_2 + margin - ||x_i - z_i + eps||_2 )

Data-parallel: N=131072 rows sharded 8 ways (16384 rows/core, no
collectives). Each core reduces its hinge terms to per-partition sums;
the host sums the 8 partials into the final scalar.

Layout: kernel() repacks x|y|z on the host into one interleaved DRAM
tensor [n_chunks, 128, 3*fd] (chunk_a=16 rows/partition/chunk) so each
chunk loads with ONE 6 MiB DMA of 48 KiB-contiguous per-partition spans
on the SP HWDGE ring. Measured floors: one big fused DMA per chunk beats
both per-tensor DMAs (fewer issues, bigger descriptors) and any
multi-ring spread (ACT/Pool-ring DMAs serialize against compute and
interleave poorly: 2-3 rings measured 15-40% slower than SP-only).

Compute reads the fused tile but writes ONLY separate tiles (ut/vt) —
keeping the DMA tile read-only is worth ~25%: in-place writes into the
load tile chain WAR hazards across chunks and stall the DMA ring.
  DVE : ut = x - y, vt = x - z            (tensor_sub into uv pool)
  ACT : rows 0..3  -> per-row Square(+eps bias), accum_out = row sum
        rows 4..15 -> one bulk Square(+eps bias) in place on ut/vt
  DVE : reduce_sum over D of rows 4..15 ([128,12,256] -> [128,12])
The two squared-distance accumulators are separate tiles (one per
writing engine) to avoid cross-engine WAW serialization.
Tail (once per pass): ACT sqrt in place, DVE hinge subtract, ACT
Relu(+margin bias) with accum_out -> per-partition sums, DMA out.

Measured (For_i repeat-count slope, 8 cores): 114.2 us/pass on a quiet
machine = 441 GB/s/core HBM read (DVE ~14.9 us/chunk is the next wall);
under heavy cross-tenant HBM contention the same config measures
~170 us, tracking the contended DMA floor.
"""

from contextlib import ExitStack

import numpy as np

import concourse.bacc as bacc
import concourse.bass as bass
import concourse.mybir as mybir
import concourse.tile as tile
from concourse import bass_utils

N_TOTAL = 131072
D = 256
N_CORES = 8
SHARD = N_TOTAL // N_CORES  # 16384 rows per core
P = 128                     # SBUF partitions
RPP = SHARD // P            # 128 rows per partition (whole shard)
CHUNK_A = 16                # rows per partition per chunk (6 MiB fused DMAs)
N_CHUNKS = RPP // CHUNK_A   # 8 chunks
FD = CHUNK_A * D            # 4096 free-dim elements per chunk tile
MARGIN = 0.5
EPS = 1e-6
F32 = mybir.dt.float32
IO_BUFS = 2   # fused [P, 3*FD] DMA tiles (48 KiB/partition each)
UV_BUFS = 3   # separate difference tiles ut/vt (16 KiB/partition each)
ACT_ROWS = 4  # rows per tensor per chunk whose square+reduce runs on ACT


def build_nc(
    repeat: int = 1,
    mode: str = "full",
    act_rows: int = ACT_ROWS,
    io_bufs: int = IO_BUFS,
    loop: bool = False,
    gp_sub: bool = False,
    chunk_a: int = CHUNK_A,
    act_dma: bool = False,
    qmap: str | None = None,
    fused: bool = True,
    fused_sep: bool = True,
    uv_bufs: int = UV_BUFS,
    fq: str = "s",
    unroll: int = 1,
) -> bass.Bass:
    """mode: 'full' | 'dma' (loads only) | 'compute' (no input loads).
    loop=True wraps the repeats in a For_i hardware loop (for timing runs
    with large repeat counts without unrolled instruction blowup)."""
    
    n_chunks = RPP // chunk_a
    fd = chunk_a * D
    nc = bacc.Bacc("TRN2", target_bir_lowering=False, debug=False)
    if fused:
        # host-repacked interleaved layout: chunk c, partition p holds that
        # chunk's x rows | y rows | z rows back to back (3*fd f32 each), so
        # one DMA per chunk loads all three tensors with maximal descriptor
        # size and minimal DMA count.
        xyz = nc.dram_tensor(
            "xyz", [n_chunks, P, 3 * fd], F32, kind="ExternalInput"
        ).ap()
    else:
        x = nc.dram_tensor("x", [SHARD, D], F32, kind="ExternalInput").ap()
        y = nc.dram_tensor("y", [SHARD, D], F32, kind="ExternalInput").ap()
        z = nc.dram_tensor("z", [SHARD, D], F32, kind="ExternalInput").ap()
    # per-partition partial hinge sums, one column per active path
    # (ACT-rows path and/or DVE-rows path)
    n_paths = (1 if act_rows else 0) + (1 if chunk_a - act_rows else 0)
    out = nc.dram_tensor("out", [P, n_paths], F32, kind="ExternalOutput").ap()

    act = mybir.ActivationFunctionType

    with tile.TileContext(nc) as tc:
        with ExitStack() as ctx:
            io = ctx.enter_context(tc.tile_pool(name="io", bufs=io_bufs))
            acc = ctx.enter_context(tc.tile_pool(name="acc", bufs=1))
            if fused_sep:
                # separate difference tiles so compute never writes into the
                # (fused) DMA tile — keeps the big load tile read-only and
                # engine work on disjoint tiles
                uv = ctx.enter_context(tc.tile_pool(name="uv", bufs=uv_bufs))

            # Per-row squared distances, split into one accumulator per
            # writing engine (a shared tile would WAW-serialize ACT vs DVE):
            #   dsq_act: written by ACT accum_out calls (act_rows per chunk)
            #   dsq_dve: written by DVE tensor_reduce   (dve_rows per chunk)
            # Each is [pos | neg] halves, matching row order between halves.
            dve_rows = chunk_a - act_rows
            na = n_chunks * act_rows   # ACT-path rows per partition
            nd = n_chunks * dve_rows   # DVE-path rows per partition
            dsq_act = acc.tile([P, max(2 * na, 1)], F32, tag="dsq_act")
            dsq_dve = acc.tile([P, max(2 * nd, 1)], F32, tag="dsq_dve")
            # per-partition hinge sums, one column per active path
            hsum = acc.tile([P, n_paths], F32, tag="hsum")

            # const bias vectors for ACT (bias must be an AP)
            eps_t = acc.tile([P, 1], F32, tag="eps")
            nc.vector.memset(eps_t[:], EPS)
            mar_t = acc.tile([P, 1], F32, tag="mar")
            nc.vector.memset(mar_t[:], MARGIN)

            if mode == "compute":
                # pre-zero both buffer slots of each io tag so compute-only
                # timing reads defined data
                tags = ("xyzt",) if fused else ("xt", "yt", "zt")
                w = 3 * fd if fused else fd
                for _ in range(io_bufs):
                    for tag in tags:
                        t = io.tile([P, w], F32, tag=tag)
                        nc.vector.memset(t[:], 0.0)

            def rep_body():
                for c in range(n_chunks):
                    rows = slice(c * P * chunk_a, (c + 1) * P * chunk_a)
                    if fused:
                        t3 = io.tile([P, 3 * fd], F32, tag="xyzt")
                        xt = t3[:, 0 * fd : 1 * fd]
                        yt = t3[:, 1 * fd : 2 * fd]
                        zt = t3[:, 2 * fd : 3 * fd]
                        if mode != "compute":
                            feng = {"s": nc.sync, "a": nc.scalar, "p": nc.gpsimd}
                            feng[fq[c % len(fq)]].dma_start(t3[:], xyz[c])
                    else:
                        xt = io.tile([P, fd], F32, tag="xt")
                        yt = io.tile([P, fd], F32, tag="yt")
                        zt = io.tile([P, fd], F32, tag="zt")
                    if not fused and mode != "compute":
                        # qmap assigns each load to an HWDGE ring by engine
                        # char: s=SP a=ACT v=DVE p=Pool t=PE.
                        #   len 3: one DMA per tensor (x, y, z)
                        #   len 6: each tensor's chunk split into two
                        #          half-tiles (x0,x1,y0,y1,z0,z1)
                        qm = qmap
                        if qm is None:
                            qm = "sas" if act_dma else "sss"
                        eng = {
                            "s": nc.sync,
                            "a": nc.scalar,
                            "v": nc.vector,
                            "p": nc.gpsimd,
                            "t": nc.tensor,
                        }
                        srcs = (x, y, z)
                        dsts = (xt, yt, zt)
                        if len(qm) == 3:
                            for q, src, dst in zip(qm, srcs, dsts):
                                eng[q].dma_start(
                                    dst[:],
                                    src[rows, :].rearrange(
                                        "(p a) d -> p (a d)", p=P
                                    ),
                                )
                        else:
                            assert len(qm) == 6
                            h = chunk_a // 2
                            for i, (src, dst) in enumerate(zip(srcs, dsts)):
                                full = src[rows, :].rearrange(
                                    "(p a) d -> p (a d)", p=P
                                )
                                for j in range(2):
                                    eng[qm[2 * i + j]].dma_start(
                                        dst[:, j * h * D : (j + 1) * h * D],
                                        full[:, j * h * D : (j + 1) * h * D],
                                    )
                    if mode == "dma":
                        continue
                    if mode == "nosq":
                        nc.vector.tensor_sub(yt[:], xt[:], yt[:])
                        nc.vector.tensor_sub(zt[:], xt[:], zt[:])
                        continue
                    if mode == "nored":
                        nc.vector.tensor_sub(yt[:], xt[:], yt[:])
                        nc.vector.tensor_sub(zt[:], xt[:], zt[:])
                        nc.scalar.activation(yt[:], yt[:], act.Square, bias=eps_t[:])
                        nc.scalar.activation(zt[:], zt[:], act.Square, bias=eps_t[:])
                        continue
                    # u = x - y in place into the y/z tiles, then (u + eps)^2
                    # on ACT (the +eps rides ACT's free bias).
                    # Per-row square+reduce is split: the first act_rows rows
                    # of each tile go through per-row ACT calls whose
                    # accum_out directly yields the row's sum; the remaining
                    # rows get one bulk ACT square + a DVE tensor_reduce.
                    if fused_sep:
                        ut = uv.tile([P, fd], F32, tag="ut")
                        vt = uv.tile([P, fd], F32, tag="vt")
                        nc.vector.tensor_sub(ut[:], xt[:], yt[:])
                        if gp_sub:
                            nc.gpsimd.tensor_sub(vt[:], xt[:], zt[:])
                        else:
                            nc.vector.tensor_sub(vt[:], xt[:], zt[:])
                        pair = ((0, ut), (1, vt))
                    else:
                        nc.vector.tensor_sub(yt[:], xt[:], yt[:])
                        if gp_sub:
                            nc.gpsimd.tensor_sub(zt[:], xt[:], zt[:])
                        else:
                            nc.vector.tensor_sub(zt[:], xt[:], zt[:])
                        pair = ((0, yt), (1, zt))
                    for half, t in pair:
                        for r in range(act_rows):
                            col = half * na + c * act_rows + r
                            nc.scalar.activation(
                                t[:, r * D : (r + 1) * D],
                                t[:, r * D : (r + 1) * D],
                                act.Square,
                                bias=eps_t[:],
                                accum_out=dsq_act[:, col : col + 1],
                            )
                        if dve_rows:
                            base = half * nd + c * dve_rows
                            nc.scalar.activation(
                                t[:, act_rows * D :],
                                t[:, act_rows * D :],
                                act.Square,
                                bias=eps_t[:],
                            )
                            nc.vector.reduce_sum(
                                dsq_dve[:, base : base + dve_rows],
                                t[:, act_rows * D :].rearrange(
                                    "p (a d) -> p a d", a=dve_rows
                                ),
                                axis=mybir.AxisListType.X,
                            )
                if mode in ("dma", "nosq", "nored"):
                    return

                # tail per accumulator: sqrt (in place), hinge with margin via
                # Relu bias, per-partition sum into its own out column
                col = 0
                for i, (dsq_t, n_cols) in enumerate(
                    ((dsq_act, na), (dsq_dve, nd))
                ):
                    if n_cols == 0:
                        continue
                    nc.scalar.activation(dsq_t[:], dsq_t[:], act.Sqrt)
                    hing = acc.tile([P, n_cols], F32, tag=f"hing{i}")
                    nc.vector.tensor_sub(
                        hing[:], dsq_t[:, :n_cols], dsq_t[:, n_cols:]
                    )
                    relu_t = acc.tile([P, n_cols], F32, tag=f"relu{i}")
                    nc.scalar.activation(
                        relu_t[:],
                        hing[:],
                        act.Relu,
                        bias=mar_t[:],
                        accum_out=hsum[:, col : col + 1],
                    )
                    col += 1
                nc.sync.dma_start(out[:], hsum[:])

            if loop and repeat > 1:
                assert repeat % unroll == 0
                with tc.For_i(0, repeat // unroll, 1):
                    for _ in range(unroll):
                        rep_body()
            else:
                for _ in range(repeat):
                    rep_body()
    nc.compile()
    return nc


def repack_fused(x, y, z, chunk_a: int = CHUNK_A) -> np.ndarray:
    """Interleave x|y|z per (chunk, partition) so each chunk is one DMA.

    Returns [N_CORES * n_chunks, P, 3 * chunk_a * D]; axis 0 shards evenly
    across cores."""
    n_chunks = RPP // chunk_a
    fd = chunk_a * D
    xr = x.reshape(N_CORES, n_chunks, P, fd)
    yr = y.reshape(N_CORES, n_chunks, P, fd)
    zr = z.reshape(N_CORES, n_chunks, P, fd)
    return np.ascontiguousarray(
        np.stack([xr, yr, zr], axis=3).reshape(N_CORES * n_chunks, P, 3 * fd)
    )


def _run(nc: bass.Bass, x, y, z, fused: bool = False, chunk_a: int = CHUNK_A):
    if fused:
        packed = repack_fused(x, y, z, chunk_a)
        n_chunks = RPP // chunk_a
        in_maps = [
            {"xyz": np.ascontiguousarray(packed[i * n_chunks : (i + 1) * n_chunks])}
            for i in range(N_CORES)
        ]
    else:
        in_maps = [
            {
                "x": np.ascontiguousarray(x[i * SHARD : (i + 1) * SHARD]),
                "y": np.ascontiguousarray(y[i * SHARD : (i + 1) * SHARD]),
                "z": np.ascontiguousarray(z[i * SHARD : (i + 1) * SHARD]),
            }
            for i in range(N_CORES)
        ]
    return bass_utils.run_bass_kernel_spmd(
        nc, in_maps, core_ids=list(range(N_CORES))
    )


_NC_CACHE = None


def kernel(x: np.ndarray, y: np.ndarray, z: np.ndarray) -> np.ndarray:
    global _NC_CACHE
    x = np.asarray(x, dtype=np.float32)
    y = np.asarray(y, dtype=np.float32)
    z = np.asarray(z, dtype=np.float32)
    if _NC_CACHE is None:
        _NC_CACHE = build_nc(1)
    res = _run(_NC_CACHE, x, y, z, fused=True, chunk_a=CHUNK_A)
    total = np.float64(0.0)
    for r in res.results:
        total += r["out"].astype(np.float64).sum()
    return np.float32(total)

